# revision 29
# baseline (speedup 1.0000x reference)
"""Multi-head attention (B=2, L=2048, D=2048, 16 heads x 128) on 8 trn2 cores.

Sharding: tensor-parallel over heads (4 groups of 4 heads) x data-parallel
over batch (2) -> 8 cores.  Each core computes, for its (batch b, group g):
    hq = q_b @ Wq_g.T, hk = kv_b @ Wk_g.T, hv = kv_b @ Wv_g.T   (4 heads)
    per head: P = softmax(hq hk^T / sqrt(128)), o = P hv
    partial_out = concat_heads(o) @ Wo[:, g].T        [2048, 2048]
Host sums the 4 per-group partials for each batch.

All matmul operands are bf16 (full PE rate, fast 97ns LDWEIGHTS vs 185ns
for f32r stationaries); accumulation stays fp32 in PSUM.  exp(scores) is
written bf16 and the softmax denominator is a bf16 elementwise add-tree on
the DVE finished by a single ones-matmul partition-fold on the PE: 49 PE
matmuls per (n, h) iteration (16 QK + 16 AV + 16 Wo + 1 fold).  The
attention mask is all-ones per the problem spec and softmax max-subtraction
is skipped (logits are O(5)).  Measured rel err ~5e-3 vs the 2e-2 gate.

Device layout notes (per core):
  qT/kvT   [2048 in, 2048 seq] bf16, host-pretransposed, streamed in blocks
  W        [128, 16 chunks x 512] bf16, host-packed so a pass's weights move
           in ~128 large DMA descriptors (row-per-partition layout)
  hqT/hkT  [128 d, 4h x 2048 seq] in SBUF (d on partitions), bf16
  hv       [128 k, 16 kt x 512(=4h x 128 d)] natural orientation, bf16
  scores^T [128 k-tile, 2x512 q] in PSUM (pp0, 3 bufs) -> exp on ACT -> bf16
  AV:      o^T[128 d, 512 q] += hv_kt.T @ exp_kt, lagged 2 exp pairs
           behind the scores (PSUM pp1, 2 bufs)
  denom:   DVE bf16 add-tree over the 16 exp chunks -> ones[128,128] fold
  Wo:      out[128 q, 1024 dout] += o_chunk.T @ woT_chunk, per q-block
"""
import math
import sys

for _p in ("/opt/trn_rl_repo", "/root/.axon_site/_ro/trn_rl_repo"):
    if _p not in sys.path:
        sys.path.append(_p)

import numpy as np

B = 2
L = 2048           # LQ == LK
DIN = 2048
NH = 16            # total heads
HL = 4             # heads per core
D = 128            # head dim
HD = HL * D        # 512, head-group width
DOUT = 2048
NC_ = 8            # cores
NCH = DIN // 128   # 16 contraction chunks
NQ = 4             # q blocks of 512
QB = 512
NKT = L // 128     # 16 key tiles

_CACHE = {}


def _build_nc():
    import concourse.bacc as bacc
    import concourse.mybir as mybir
    import concourse.tile as tile

    F32R = mybir.dt.float32r
    F32 = mybir.dt.float32
    BF16 = mybir.dt.bfloat16

    nc = bacc.Bacc("TRN2", target_bir_lowering=False, debug=False)
    # q/kv/w streams in bf16: halves the phase-1 HBM traffic (pass starts
    # were over the 360 GB/s aggregate DMA roof in f32).
    qT = nc.dram_tensor("qT", [DIN, L], BF16, kind="ExternalInput").ap()
    kvT = nc.dram_tensor("kvT", [DIN, L], BF16, kind="ExternalInput").ap()
    # weights arrive pre-packed [128, NCH*HD]: partition p holds chunk rows
    # (c*128+p, :) back to back, so a whole pass's W moves in ~128 large DMA
    # descriptors (the 2048 small-row descriptors were throttling the DMA
    # engines at pass starts).
    wqT = nc.dram_tensor("wqT", [128, NCH * HD], BF16, kind="ExternalInput").ap()
    wkT = nc.dram_tensor("wkT", [128, NCH * HD], BF16, kind="ExternalInput").ap()
    wvT = nc.dram_tensor("wvT", [128, NCH * HD], BF16, kind="ExternalInput").ap()
    woT = nc.dram_tensor("woT", [HD, DOUT], BF16, kind="ExternalInput").ap()
    out = nc.dram_tensor("out", [L, DOUT], F32R, kind="ExternalOutput").ap()

    EXP = mybir.ActivationFunctionType.Exp
    COPY = mybir.ActivationFunctionType.Copy

    with tile.TileContext(nc) as tc:
        with (
            nc.allow_low_precision(reason="f32r/bf16 attention, tol 2e-2"),
            tc.tile_pool(name="persist", bufs=1) as pp,
            tc.tile_pool(name="psum", bufs=2, space="PSUM") as psp,
        ):
            hq_sb = pp.tile([128, HL * L], BF16, tag="hq")
            hk_sb = pp.tile([128, HL * L], BF16, tag="hk")
            hv_sb = pp.tile([128, NKT * HD], BF16, tag="hv")
            ones_sb = pp.tile([128, 128], BF16, tag="ones")
            nc.gpsimd.memset(ones_sb[:], 1.0)

            # ---------------- projections ----------------
            with tc.tile_pool(name="proj", bufs=1) as jp:
                for pass_i, (w_dram, x_dram, dst) in enumerate(
                    [(wqT, qT, hq_sb), (wkT, kvT, hk_sb), (wvT, kvT, hv_sb)]
                ):
                    is_v = pass_i == 2
                    # one packed W tile per pass.  Pass 0 loads chunks 0-3 on
                    # the sync (HWDGE) queue ahead of the first x chunks (the
                    # gpsimd SWDGE path has ~4us trigger-to-data latency that
                    # would gate the very first matmul); the rest ride gpsimd
                    # in 4-chunk slabs.
                    w_sb = jp.tile(
                        [128, NCH * HD], BF16, tag="w", bufs=2, name=f"w{pass_i}"
                    )
                    if pass_i == 0:
                        # pass 0 loads ALL weights on the sync (HWDGE) queue,
                        # interleaved with the x stream in consumption order:
                        # one queue means the big weight descriptors never
                        # race the x descriptors at the DMA engines (the
                        # gpsimd queue's descs interleave 1:1 per engine and
                        # were delaying the warmup x blocks ~3us each).
                        nc.sync.dma_start(
                            out=w_sb[:, 0 : 2 * HD], in_=w_dram[:, 0 : 2 * HD]
                        )
                    for cs4 in range(4) if pass_i > 0 else []:
                        sl = slice(cs4 * 4 * HD, (cs4 + 1) * 4 * HD)
                        nc.gpsimd.dma_start(out=w_sb[:, sl], in_=w_dram[:, sl])
                    for n in range(NQ):
                        if is_v and n == NQ - 1:
                            # last projection block: put j0/j1 in pp1 (idle
                            # until attention) so phase 2's first score tiles
                            # only wait on ONE pp0 drain at the transition
                            accJ0 = psp.tile([128, QB], F32, tag="pp1", bufs=2, name="vj0")
                            accJ1 = psp.tile([128, QB], F32, tag="pp1", bufs=2, name="vj1")
                            accC = psp.tile([128, 2 * QB], F32, tag="pp0", bufs=3, name="accC")
                            accs = [
                                accJ0[:],
                                accJ1[:],
                                accC[:, 0:QB],
                                accC[:, QB : 2 * QB],
                            ]
                        else:
                            accA = psp.tile([128, 2 * QB], F32, tag="pp0", bufs=3, name="accA")
                            accB = psp.tile([128, 2 * QB], F32, tag="pp0", bufs=3, name="accB")
                            accs = [
                                accA[:, 0:QB],
                                accB[:, 0:QB],
                                accB[:, QB : 2 * QB],
                                accA[:, QB : 2 * QB],
                            ]
                        for cs in range(NCH // 4):
                            # 512KB super-block: 4 contraction chunks per DMA
                            sblk = jp.tile([128, 4 * QB], BF16, tag="blk", bufs=5, name="sblk")
                            if pass_i == 0 and n == 0 and cs == 0:
                                # chunk-split the first x block so matmul 0
                                # only waits on a ~128KB transfer; w chunks
                                # 2-3 slot in after the first x chunk
                                for ci in range(4):
                                    nc.sync.dma_start(
                                        out=sblk[:, ci * QB : (ci + 1) * QB],
                                        in_=x_dram[
                                            ci * 128 : (ci + 1) * 128, 0:QB
                                        ],
                                    )
                                    if ci == 0:
                                        nc.sync.dma_start(
                                            out=w_sb[:, 2 * HD : 4 * HD],
                                            in_=w_dram[:, 2 * HD : 4 * HD],
                                        )
                            else:
                                nc.sync.dma_start(
                                    out=sblk.rearrange("p (c q) -> p c q", q=QB),
                                    in_=x_dram[
                                        cs * 512 : (cs + 1) * 512, n * QB : (n + 1) * QB
                                    ].rearrange("(c p) q -> p c q", p=128),
                                )
                                if pass_i == 0 and n == 0 and cs >= 1:
                                    # w slab for chunks 4cs..4cs+3 right after
                                    # the x block that precedes its first use
                                    wsl = slice(cs * 4 * HD, (cs + 1) * 4 * HD)
                                    nc.sync.dma_start(
                                        out=w_sb[:, wsl], in_=w_dram[:, wsl]
                                    )
                            for ci in range(4):
                                c = cs * 4 + ci
                                blk = sblk[:, ci * QB : (ci + 1) * QB]
                                for j in range(4):
                                    if is_v:
                                        # hv[k, d]: lhsT = kv block cols, rhs = w chunk
                                        nc.tensor.matmul(
                                            accs[j][:],
                                            blk[:, j * 128 : (j + 1) * 128],
                                            w_sb[:, c * HD : (c + 1) * HD],
                                            start=(c == 0),
                                            stop=(c == NCH - 1),
                                        )
                                    else:
                                        # hxT[d, q]: lhsT = w chunk head j, rhs = x block
                                        nc.tensor.matmul(
                                            accs[j][:],
                                            w_sb[:, c * HD + j * 128 : c * HD + (j + 1) * 128],
                                            blk[:],
                                            start=(c == 0),
                                            stop=(c == NCH - 1),
                                        )
                        last_blk = is_v and n == NQ - 1
                        for j in range(4):
                            if is_v:
                                dst_sl = dst[:, (n * 4 + j) * HD : (n * 4 + j + 1) * HD]
                            else:
                                dst_sl = dst[:, j * L + n * QB : j * L + (n + 1) * QB]
                            if last_blk and j in (0, 1):
                                # pp1 tiles drain on DVE (frees them for the
                                # first AV accumulators); accC's j2/j3 drain
                                # on ACT in parallel
                                nc.vector.tensor_copy(out=dst_sl, in_=accs[j][:])
                            else:
                                nc.scalar.activation(dst_sl, accs[j][:], COPY)

            # ---------------- attention + Wo ----------------
            with tc.tile_pool(name="attn", bufs=1) as ap:
                wo_sb = ap.tile([128, HL * DOUT], BF16, tag="wo", bufs=1, name="wo")
                for h in range(HL):
                    nc.gpsimd.dma_start(
                        out=wo_sb[:, h * DOUT : (h + 1) * DOUT],
                        in_=woT[h * 128 : (h + 1) * 128, :],
                    )

                def flush(st):
                    # normalize the previous (n, h): partition-fold the bf16
                    # denominator tree with a ones-matmul, reciprocal, scale.
                    _, h_, ps_o_, dtree_, o_sb_ = st
                    ps_d = psp.tile([128, 2 * QB], F32, tag="pp0", bufs=3, name="ps_d")
                    nc.tensor.matmul(
                        ps_d[:, 0:QB], ones_sb[:], dtree_[:, 0:QB], start=True, stop=True
                    )
                    recip = ap.tile([128, QB], F32, tag="recip", bufs=2, name="recip")
                    nc.vector.reciprocal_approx_fast(out=recip[:], in_=ps_d[:, 0:QB])
                    nc.vector.tensor_mul(
                        out=o_sb_[:, h_ * QB : (h_ + 1) * QB],
                        in0=ps_o_[:],
                        in1=recip[:],
                    )

                def emit_wo_group(n_, o_sb_, g, on_act=False):
                    # one Wo output group (qtl, mp) for q block n_: 8 matmuls
                    qtl, mp = divmod(g, 2)
                    ps_f = psp.tile([128, 2 * QB], F32, tag="pp0", bufs=3, name="ps_f")
                    for h_ in range(HL):
                        for t in range(2):
                            m = 2 * mp + t
                            nc.tensor.matmul(
                                ps_f[:, t * QB : (t + 1) * QB],
                                o_sb_[:, h_ * QB + qtl * 128 : h_ * QB + (qtl + 1) * 128],
                                wo_sb[:, h_ * DOUT + m * QB : h_ * DOUT + (m + 1) * QB],
                                start=(h_ == 0),
                                stop=(h_ == HL - 1),
                            )
                    stage = ap.tile([128, 2 * QB], F32R, tag="stage", bufs=3, name="stage")
                    if on_act:
                        nc.scalar.activation(stage[:], ps_f[:], COPY)
                    else:
                        # split the PSUM drain across DVE+ACT halves: the pp0
                        # tile frees in ~0.66us instead of 1.2us, so the score
                        # tile three allocations later never stalls on it.
                        # The ACT half lands in ACT's catch-up window.
                        nc.vector.tensor_copy(
                            out=stage[:, 0:QB], in_=ps_f[:, 0:QB]
                        )
                        nc.scalar.activation(
                            stage[:, QB : 2 * QB], ps_f[:, QB : 2 * QB], COPY
                        )
                    nc.sync.dma_start(
                        out=out[
                            n_ * QB + qtl * 128 : n_ * QB + (qtl + 1) * 128,
                            mp * 2 * QB : (mp + 1) * 2 * QB,
                        ],
                        in_=stage[:],
                    )

                pending = None
                o_tiles = {}
                for n in range(NQ):
                    o_sb = ap.tile([128, HL * QB], BF16, tag="o", bufs=2, name="o")
                    o_tiles[n] = o_sb
                    for h in range(HL):
                        hq_sl = hq_sb[:, h * L + n * QB : h * L + (n + 1) * QB]
                        ps_o = psp.tile([128, QB], F32, tag="pp1", bufs=2, name="ps_o")
                        dtree = ap.tile(
                            [128, 8 * QB], BF16, tag="dtree", bufs=2, name="dtree"
                        )
                        exp_half = [None, None]
                        # 10 cycles: scores/exp for pair p, AV lagged TWO
                        # pairs behind -- exp takes ~1.6us from the pair's
                        # scores while the PE reaches a 1-lag AV in ~1.4us,
                        # so lag 1 stalled ~0.2us per pair.  The DVE add-tree
                        # tracks the exp chunks as they land.
                        for p in range(10):
                            if p < 8:
                                half = p // 4
                                if p % 4 == 0:
                                    exp_half[half] = ap.tile(
                                        [128, 8 * QB], BF16, tag="exp", bufs=3, name="exp"
                                    )
                                eh = exp_half[half]
                                off = (p % 4) * 2 * QB
                                ps_s = psp.tile(
                                    [128, 2 * QB], F32, tag="pp0", bufs=3, name="ps_s"
                                )
                                for t in range(2):
                                    kt = 2 * p + t
                                    nc.tensor.matmul(
                                        ps_s[:, t * QB : (t + 1) * QB],
                                        hk_sb[:, h * L + kt * 128 : h * L + (kt + 1) * 128],
                                        hq_sl,
                                        start=True,
                                        stop=True,
                                    )
                                nc.scalar.activation(
                                    eh[:, off : off + 2 * QB], ps_s[:], EXP
                                )
                                # bf16 denominator tree on DVE (in dtree):
                                # half h: A=ch01+ch23, B=ch45+ch67, C=A+B.
                                # A right after its exps; B/C deferred one
                                # p-block so the p3 stage CAST isn't queued
                                # behind adds still waiting on later exps.
                                hb = half * 4 * QB
                                if p % 4 == 1:
                                    nc.vector.tensor_add(
                                        out=dtree[:, hb : hb + 2 * QB],
                                        in0=eh[:, 0 : 2 * QB],
                                        in1=eh[:, 2 * QB : 4 * QB],
                                    )
                            if p in (4, 8):
                                hb = (p // 4 - 1) * 4 * QB
                                eh = exp_half[p // 4 - 1]
                                nc.vector.tensor_add(
                                    out=dtree[:, hb + 2 * QB : hb + 4 * QB],
                                    in0=eh[:, 4 * QB : 6 * QB],
                                    in1=eh[:, 6 * QB : 8 * QB],
                                )
                                nc.vector.tensor_add(
                                    out=dtree[:, hb : hb + 2 * QB],
                                    in0=dtree[:, hb : hb + 2 * QB],
                                    in1=dtree[:, hb + 2 * QB : hb + 4 * QB],
                                )
                            if p >= 2:
                                for t in range(2):
                                    kt = 2 * (p - 2) + t
                                    e_sl = exp_half[kt // 8][
                                        :, (kt % 8) * QB : (kt % 8 + 1) * QB
                                    ]
                                    nc.tensor.matmul(
                                        ps_o[:],
                                        hv_sb[:, kt * HD + h * 128 : kt * HD + (h + 1) * 128],
                                        e_sl,
                                        start=(kt == 0),
                                        stop=(kt == NKT - 1),
                                    )
                            if p == 2 and pending is not None:
                                # flush early: recip then frees ps_d's PSUM
                                # slot well before the pp0 rotation reuses it,
                                # and o_sb is fully normalized before any Wo
                                # group of this h reads it.
                                flush(pending)
                                pending = None
                            if p == 3 and n > 0:
                                emit_wo_group(n - 1, o_tiles[n - 1], 2 * h)
                        # finish the tree (D = C0+C1, E = D.lo + D.hi) BEFORE
                        # the second stage copy queues on the DVE, so the next
                        # flush's fold input is ready early.
                        nc.vector.tensor_add(
                            out=dtree[:, 0 : 2 * QB],
                            in0=dtree[:, 0 : 2 * QB],
                            in1=dtree[:, 4 * QB : 6 * QB],
                        )
                        nc.vector.tensor_add(
                            out=dtree[:, 0:QB],
                            in0=dtree[:, 0:QB],
                            in1=dtree[:, QB : 2 * QB],
                        )
                        if n > 0:
                            # very last per-iter group: stage on ACT (no exps
                            # left) so the DVE finishes D/E for the final
                            # flush without a 1.2us CAST in front
                            emit_wo_group(
                                n - 1,
                                o_tiles[n - 1],
                                2 * h + 1,
                                on_act=(n == NQ - 1 and h == HL - 1),
                            )
                        pending = (n, h, ps_o, dtree, o_sb)
                    if n > 0:
                        o_tiles.pop(n - 1)
                flush(pending)
                o_last = o_tiles.pop(NQ - 1)
                for g in range(8):
                    # alternate the stage copies ACT/DVE so the final 8
                    # groups' drains pipeline two wide (ACT has no exp left)
                    emit_wo_group(NQ - 1, o_last, g, on_act=(g % 2 == 1))
    nc.compile()
    return nc


def _get_nc():
    if "nc" not in _CACHE:
        _CACHE["nc"] = _build_nc()
    return _CACHE["nc"]


def make_in_maps(query, key_value, Wq, Wk, Wv, Wo):
    import ml_dtypes

    scale = 1.0 / math.sqrt(D)
    f32 = np.float32
    bf16 = ml_dtypes.bfloat16

    def pack_w(w_sl):
        # [HD, DIN] weight slice -> transposed [DIN, HD] -> packed
        # [128, NCH*HD]: partition p row = concat_c w.T[c*128+p, :]
        wT = w_sl.T.astype(bf16)
        return np.ascontiguousarray(
            wT.reshape(NCH, 128, HD).transpose(1, 0, 2).reshape(128, NCH * HD)
        )

    in_maps = []
    qT = [np.ascontiguousarray(query[b].T.astype(bf16)) for b in range(B)]
    kvT = [np.ascontiguousarray(key_value[b].T.astype(bf16)) for b in range(B)]
    for core in range(NC_):
        b, g = divmod(core, NC_ // B)
        sl = slice(g * HD, (g + 1) * HD)
        in_maps.append(
            {
                "qT": qT[b],
                "kvT": kvT[b],
                "wqT": pack_w(Wq[sl, :].astype(f32) * scale),
                "wkT": pack_w(Wk[sl, :]),
                "wvT": pack_w(Wv[sl, :]),
                "woT": np.ascontiguousarray(Wo[:, sl].T.astype(bf16)),
            }
        )
    return in_maps


def _numpy_fallback(query, key_value, attention_mask, Wq, Wk, Wv, Wo):
    # Only reached if the mask is not all-ones (never per the problem spec).
    q64, kv64 = query.astype(np.float64), key_value.astype(np.float64)
    hq = (q64 @ Wq.T.astype(np.float64)).reshape(B, L, NH, D).transpose(0, 2, 1, 3)
    hk = (kv64 @ Wk.T.astype(np.float64)).reshape(B, L, NH, D).transpose(0, 2, 1, 3)
    hv = (kv64 @ Wv.T.astype(np.float64)).reshape(B, L, NH, D).transpose(0, 2, 1, 3)
    s = np.einsum("bhqd,bhkd->bhqk", hq, hk) / math.sqrt(D)
    mask = attention_mask[:, None, :, :]
    s = np.where(mask, s, -np.inf)
    s = s - s.max(axis=-1, keepdims=True)
    e = np.exp(s)
    p = e / np.maximum(e.sum(axis=-1, keepdims=True), 1e-300)
    p = np.where(mask, p, 0.0)
    o = np.einsum("bhqk,bhkd->bhqd", p, hv)
    o = o.transpose(0, 2, 1, 3).reshape(B, L, NH * D)
    return (o @ Wo.T.astype(np.float64)).astype(np.float32)


def kernel(query, key_value, attention_mask, Wq, Wk, Wv, Wo):
    query = np.asarray(query)
    key_value = np.asarray(key_value)
    attention_mask = np.asarray(attention_mask)
    Wq, Wk, Wv, Wo = (np.asarray(a) for a in (Wq, Wk, Wv, Wo))

    if not attention_mask.all():
        return _numpy_fallback(query, key_value, attention_mask, Wq, Wk, Wv, Wo)

    from concourse.bass_utils import run_bass_kernel_spmd

    nc = _get_nc()
    in_maps = make_in_maps(query, key_value, Wq, Wk, Wv, Wo)
    res = run_bass_kernel_spmd(nc, in_maps, list(range(NC_))).results
    out = np.zeros((B, L, DOUT), np.float32)
    for core in range(NC_):
        b = core // (NC_ // B)
        out[b] += res[core]["out"]
    return out


# revision 32
# speedup vs baseline: 1.0265x; 1.0265x over previous
"""Multi-head attention (B=2, L=2048, D=2048, 16 heads x 128) on 8 trn2 cores.

Sharding: tensor-parallel over heads (4 groups of 4 heads) x data-parallel
over batch (2) -> 8 cores.  Each core computes, for its (batch b, group g):
    hq = q_b @ Wq_g.T, hk = kv_b @ Wk_g.T, hv = kv_b @ Wv_g.T   (4 heads)
    per head: P = softmax(hq hk^T / sqrt(128)), o = P hv
    partial_out = concat_heads(o) @ Wo[:, g].T        [2048, 2048]
Host sums the 4 per-group partials for each batch.

All matmul operands are bf16 (full PE rate, fast 97ns LDWEIGHTS vs 185ns
for f32r stationaries); accumulation stays fp32 in PSUM.  exp(scores) is
written bf16 and the softmax denominator is a bf16 elementwise add-tree on
the DVE finished by a single ones-matmul partition-fold on the PE: 49 PE
matmuls per (n, h) iteration (16 QK + 16 AV + 16 Wo + 1 fold).  The
attention mask is all-ones per the problem spec and softmax max-subtraction
is skipped (logits are O(5)).  Measured rel err ~5e-3 vs the 2e-2 gate.

Device layout notes (per core):
  qT/kvT   [2048 in, 2048 seq] bf16, host-pretransposed, streamed in blocks
  W        [128, 16 chunks x 512] bf16, host-packed so a pass's weights move
           in ~128 large DMA descriptors (row-per-partition layout)
  hqT/hkT  [128 d, 4h x 2048 seq] in SBUF (d on partitions), bf16
  hv       [128 k, 16 kt x 512(=4h x 128 d)] natural orientation, bf16
  scores^T [128 k-tile, 2x512 q] in PSUM (pp0, 3 bufs) -> exp on ACT -> bf16
  AV:      o^T[128 d, 512 q] += hv_kt.T @ exp_kt, lagged 2 exp pairs
           behind the scores (PSUM pp1, 2 bufs)
  denom:   DVE bf16 add-tree over the 16 exp chunks -> ones[128,128] fold
  Wo:      out[128 q, 1024 dout] += o_chunk.T @ woT_chunk, per q-block
"""
import math
import sys

for _p in ("/opt/trn_rl_repo", "/root/.axon_site/_ro/trn_rl_repo"):
    if _p not in sys.path:
        sys.path.append(_p)

import numpy as np

B = 2
L = 2048           # LQ == LK
DIN = 2048
NH = 16            # total heads
HL = 4             # heads per core
D = 128            # head dim
HD = HL * D        # 512, head-group width
DOUT = 2048
NC_ = 8            # cores
NCH = DIN // 128   # 16 contraction chunks
NQ = 4             # q blocks of 512
QB = 512
NKT = L // 128     # 16 key tiles

_CACHE = {}


def _build_nc():
    import concourse.bacc as bacc
    import concourse.mybir as mybir
    import concourse.tile as tile

    F32R = mybir.dt.float32r
    F32 = mybir.dt.float32
    BF16 = mybir.dt.bfloat16

    nc = bacc.Bacc("TRN2", target_bir_lowering=False, debug=False)
    # q/kv/w streams in bf16: halves the phase-1 HBM traffic (pass starts
    # were over the 360 GB/s aggregate DMA roof in f32).
    qT = nc.dram_tensor("qT", [DIN, L], BF16, kind="ExternalInput").ap()
    kvT = nc.dram_tensor("kvT", [DIN, L], BF16, kind="ExternalInput").ap()
    # weights arrive pre-packed [128, NCH*HD]: partition p holds chunk rows
    # (c*128+p, :) back to back, so a whole pass's W moves in ~128 large DMA
    # descriptors (the 2048 small-row descriptors were throttling the DMA
    # engines at pass starts).
    wqT = nc.dram_tensor("wqT", [128, NCH * HD], BF16, kind="ExternalInput").ap()
    wkT = nc.dram_tensor("wkT", [128, NCH * HD], BF16, kind="ExternalInput").ap()
    wvT = nc.dram_tensor("wvT", [128, NCH * HD], BF16, kind="ExternalInput").ap()
    woT = nc.dram_tensor("woT", [HD, DOUT], BF16, kind="ExternalInput").ap()
    out = nc.dram_tensor("out", [L, DOUT], F32R, kind="ExternalOutput").ap()

    EXP = mybir.ActivationFunctionType.Exp
    COPY = mybir.ActivationFunctionType.Copy

    with tile.TileContext(nc) as tc:
        with (
            nc.allow_low_precision(reason="f32r/bf16 attention, tol 2e-2"),
            tc.tile_pool(name="persist", bufs=1) as pp,
            tc.tile_pool(name="psum", bufs=2, space="PSUM") as psp,
        ):
            hq_sb = pp.tile([128, HL * L], BF16, tag="hq")
            hk_sb = pp.tile([128, HL * L], BF16, tag="hk")
            hv_sb = pp.tile([128, NKT * HD], BF16, tag="hv")
            ones_sb = pp.tile([128, 128], BF16, tag="ones")
            nc.gpsimd.memset(ones_sb[:], 1.0)

            # ---------------- projections ----------------
            with tc.tile_pool(name="proj", bufs=1) as jp:
                for pass_i, (w_dram, x_dram, dst) in enumerate(
                    [(wqT, qT, hq_sb), (wkT, kvT, hk_sb), (wvT, kvT, hv_sb)]
                ):
                    is_v = pass_i == 2
                    # one packed W tile per pass.  Pass 0 loads chunks 0-3 on
                    # the sync (HWDGE) queue ahead of the first x chunks (the
                    # gpsimd SWDGE path has ~4us trigger-to-data latency that
                    # would gate the very first matmul); the rest ride gpsimd
                    # in 4-chunk slabs.
                    w_sb = jp.tile(
                        [128, NCH * HD], BF16, tag="w", bufs=2, name=f"w{pass_i}"
                    )
                    if pass_i == 0:
                        # pass 0 loads ALL weights on the sync (HWDGE) queue,
                        # interleaved with the x stream in consumption order:
                        # one queue means the big weight descriptors never
                        # race the x descriptors at the DMA engines (the
                        # gpsimd queue's descs interleave 1:1 per engine and
                        # were delaying the warmup x blocks ~3us each).
                        nc.sync.dma_start(
                            out=w_sb[:, 0 : 2 * HD], in_=w_dram[:, 0 : 2 * HD]
                        )
                    for cs4 in range(4) if pass_i > 0 else []:
                        sl = slice(cs4 * 4 * HD, (cs4 + 1) * 4 * HD)
                        nc.gpsimd.dma_start(out=w_sb[:, sl], in_=w_dram[:, sl])
                    for n in range(NQ):
                        if is_v and n == NQ - 1:
                            # last projection block: put j0/j1 in pp1 (idle
                            # until attention) so phase 2's first score tiles
                            # only wait on ONE pp0 drain at the transition
                            accJ0 = psp.tile([128, QB], F32, tag="pp1", bufs=2, name="vj0")
                            accJ1 = psp.tile([128, QB], F32, tag="pp1", bufs=2, name="vj1")
                            accC = psp.tile([128, 2 * QB], F32, tag="pp0", bufs=3, name="accC")
                            accs = [
                                accJ0[:],
                                accJ1[:],
                                accC[:, 0:QB],
                                accC[:, QB : 2 * QB],
                            ]
                        else:
                            accA = psp.tile([128, 2 * QB], F32, tag="pp0", bufs=3, name="accA")
                            accB = psp.tile([128, 2 * QB], F32, tag="pp0", bufs=3, name="accB")
                            accs = [
                                accA[:, 0:QB],
                                accB[:, 0:QB],
                                accB[:, QB : 2 * QB],
                                accA[:, QB : 2 * QB],
                            ]
                        for cs in range(NCH // 4):
                            # 512KB super-block: 4 contraction chunks per DMA
                            sblk = jp.tile([128, 4 * QB], BF16, tag="blk", bufs=5, name="sblk")
                            if pass_i == 0 and n == 0 and cs == 0:
                                # chunk-split the first x block so matmul 0
                                # only waits on a ~128KB transfer; w chunks
                                # 2-3 slot in after the first x chunk
                                for ci in range(4):
                                    nc.sync.dma_start(
                                        out=sblk[:, ci * QB : (ci + 1) * QB],
                                        in_=x_dram[
                                            ci * 128 : (ci + 1) * 128, 0:QB
                                        ],
                                    )
                                    if ci == 1:
                                        # chunk 1 only needs the w01 slab, so
                                        # xc1 jumps ahead of the w23 load
                                        nc.sync.dma_start(
                                            out=w_sb[:, 2 * HD : 4 * HD],
                                            in_=w_dram[:, 2 * HD : 4 * HD],
                                        )
                            else:
                                nc.sync.dma_start(
                                    out=sblk.rearrange("p (c q) -> p c q", q=QB),
                                    in_=x_dram[
                                        cs * 512 : (cs + 1) * 512, n * QB : (n + 1) * QB
                                    ].rearrange("(c p) q -> p c q", p=128),
                                )
                                if pass_i == 0 and n == 0 and cs >= 1:
                                    # w slab for chunks 4cs..4cs+3 right after
                                    # the x block that precedes its first use
                                    wsl = slice(cs * 4 * HD, (cs + 1) * 4 * HD)
                                    nc.sync.dma_start(
                                        out=w_sb[:, wsl], in_=w_dram[:, wsl]
                                    )
                            for ci in range(4):
                                c = cs * 4 + ci
                                blk = sblk[:, ci * QB : (ci + 1) * QB]
                                for j in range(4):
                                    if is_v:
                                        # hv[k, d]: lhsT = kv block cols, rhs = w chunk
                                        nc.tensor.matmul(
                                            accs[j][:],
                                            blk[:, j * 128 : (j + 1) * 128],
                                            w_sb[:, c * HD : (c + 1) * HD],
                                            start=(c == 0),
                                            stop=(c == NCH - 1),
                                        )
                                    else:
                                        # hxT[d, q]: lhsT = w chunk head j, rhs = x block
                                        nc.tensor.matmul(
                                            accs[j][:],
                                            w_sb[:, c * HD + j * 128 : c * HD + (j + 1) * 128],
                                            blk[:],
                                            start=(c == 0),
                                            stop=(c == NCH - 1),
                                        )
                        last_blk = is_v and n == NQ - 1
                        # last block: drain the pp0 accumulator (j2/j3) FIRST
                        # -- it gates phase 2's third score tile -- with j2 on
                        # DVE and j3 on ACT in parallel; then j0/j1 on DVE so
                        # ACT is free for the first exp immediately after j3.
                        for j in [2, 3, 0, 1] if last_blk else range(4):
                            if is_v:
                                dst_sl = dst[:, (n * 4 + j) * HD : (n * 4 + j + 1) * HD]
                            else:
                                dst_sl = dst[:, j * L + n * QB : j * L + (n + 1) * QB]
                            if last_blk and j != 3:
                                nc.vector.tensor_copy(out=dst_sl, in_=accs[j][:])
                            else:
                                nc.scalar.activation(dst_sl, accs[j][:], COPY)

            # ---------------- attention + Wo ----------------
            with tc.tile_pool(name="attn", bufs=1) as ap:
                wo_sb = ap.tile([128, HL * DOUT], BF16, tag="wo", bufs=1, name="wo")
                for h in range(HL):
                    nc.gpsimd.dma_start(
                        out=wo_sb[:, h * DOUT : (h + 1) * DOUT],
                        in_=woT[h * 128 : (h + 1) * 128, :],
                    )

                def flush(st):
                    # normalize the previous (n, h): partition-fold the bf16
                    # denominator tree with a ones-matmul, reciprocal, scale.
                    _, h_, ps_o_, dtree_, o_sb_ = st
                    ps_d = psp.tile([128, 2 * QB], F32, tag="pp0", bufs=3, name="ps_d")
                    nc.tensor.matmul(
                        ps_d[:, 0:QB], ones_sb[:], dtree_[:, 0:QB], start=True, stop=True
                    )
                    recip = ap.tile([128, QB], F32, tag="recip", bufs=2, name="recip")
                    nc.vector.reciprocal_approx_fast(out=recip[:], in_=ps_d[:, 0:QB])
                    nc.vector.tensor_mul(
                        out=o_sb_[:, h_ * QB : (h_ + 1) * QB],
                        in0=ps_o_[:],
                        in1=recip[:],
                    )

                def emit_wo_group(n_, o_sb_, g, on_act=False):
                    # one Wo output group (qtl, mp) for q block n_: 8 matmuls
                    qtl, mp = divmod(g, 2)
                    ps_f = psp.tile([128, 2 * QB], F32, tag="pp0", bufs=3, name="ps_f")
                    for h_ in range(HL):
                        for t in range(2):
                            m = 2 * mp + t
                            nc.tensor.matmul(
                                ps_f[:, t * QB : (t + 1) * QB],
                                o_sb_[:, h_ * QB + qtl * 128 : h_ * QB + (qtl + 1) * 128],
                                wo_sb[:, h_ * DOUT + m * QB : h_ * DOUT + (m + 1) * QB],
                                start=(h_ == 0),
                                stop=(h_ == HL - 1),
                            )
                    stage = ap.tile([128, 2 * QB], F32R, tag="stage", bufs=3, name="stage")
                    if on_act:
                        nc.scalar.activation(stage[:], ps_f[:], COPY)
                    else:
                        nc.vector.tensor_copy(out=stage[:], in_=ps_f[:])
                    nc.sync.dma_start(
                        out=out[
                            n_ * QB + qtl * 128 : n_ * QB + (qtl + 1) * 128,
                            mp * 2 * QB : (mp + 1) * 2 * QB,
                        ],
                        in_=stage[:],
                    )

                pending = None
                o_tiles = {}
                for n in range(NQ):
                    o_sb = ap.tile([128, HL * QB], BF16, tag="o", bufs=2, name="o")
                    o_tiles[n] = o_sb
                    for h in range(HL):
                        hq_sl = hq_sb[:, h * L + n * QB : h * L + (n + 1) * QB]
                        ps_o = psp.tile([128, QB], F32, tag="pp1", bufs=2, name="ps_o")
                        dtree = ap.tile(
                            [128, 8 * QB], BF16, tag="dtree", bufs=2, name="dtree"
                        )
                        exp_half = [None, None]
                        # 10 cycles: scores/exp for pair p, AV lagged TWO
                        # pairs behind -- exp takes ~1.6us from the pair's
                        # scores while the PE reaches a 1-lag AV in ~1.4us,
                        # so lag 1 stalled ~0.2us per pair.  The DVE add-tree
                        # tracks the exp chunks as they land.
                        for p in range(10):
                            if p < 8:
                                half = p // 4
                                if p % 4 == 0:
                                    exp_half[half] = ap.tile(
                                        [128, 8 * QB], BF16, tag="exp", bufs=3, name="exp"
                                    )
                                eh = exp_half[half]
                                off = (p % 4) * 2 * QB
                                ps_s = psp.tile(
                                    [128, 2 * QB], F32, tag="pp0", bufs=3, name="ps_s"
                                )
                                for t in range(2):
                                    kt = 2 * p + t
                                    nc.tensor.matmul(
                                        ps_s[:, t * QB : (t + 1) * QB],
                                        hk_sb[:, h * L + kt * 128 : h * L + (kt + 1) * 128],
                                        hq_sl,
                                        start=True,
                                        stop=True,
                                    )
                                nc.scalar.activation(
                                    eh[:, off : off + 2 * QB], ps_s[:], EXP
                                )
                                # bf16 denominator tree on DVE (in dtree):
                                # half h: A=ch01+ch23, B=ch45+ch67, C=A+B.
                                # A right after its exps; B/C deferred one
                                # p-block so the p3 stage CAST isn't queued
                                # behind adds still waiting on later exps.
                                hb = half * 4 * QB
                                if p % 4 == 1:
                                    nc.vector.tensor_add(
                                        out=dtree[:, hb : hb + 2 * QB],
                                        in0=eh[:, 0 : 2 * QB],
                                        in1=eh[:, 2 * QB : 4 * QB],
                                    )
                            if p in (4, 8):
                                hb = (p // 4 - 1) * 4 * QB
                                eh = exp_half[p // 4 - 1]
                                nc.vector.tensor_add(
                                    out=dtree[:, hb + 2 * QB : hb + 4 * QB],
                                    in0=eh[:, 4 * QB : 6 * QB],
                                    in1=eh[:, 6 * QB : 8 * QB],
                                )
                                nc.vector.tensor_add(
                                    out=dtree[:, hb : hb + 2 * QB],
                                    in0=dtree[:, hb : hb + 2 * QB],
                                    in1=dtree[:, hb + 2 * QB : hb + 4 * QB],
                                )
                            if p >= 2:
                                for t in range(2):
                                    kt = 2 * (p - 2) + t
                                    e_sl = exp_half[kt // 8][
                                        :, (kt % 8) * QB : (kt % 8 + 1) * QB
                                    ]
                                    nc.tensor.matmul(
                                        ps_o[:],
                                        hv_sb[:, kt * HD + h * 128 : kt * HD + (h + 1) * 128],
                                        e_sl,
                                        start=(kt == 0),
                                        stop=(kt == NKT - 1),
                                    )
                            if p == 2 and pending is not None:
                                # flush early: recip then frees ps_d's PSUM
                                # slot well before the pp0 rotation reuses it,
                                # and o_sb is fully normalized before any Wo
                                # group of this h reads it.
                                flush(pending)
                                pending = None
                            if p == 3 and n > 0:
                                emit_wo_group(n - 1, o_tiles[n - 1], 2 * h)
                        # finish the tree (D = C0+C1, E = D.lo + D.hi) BEFORE
                        # the second stage copy queues on the DVE, so the next
                        # flush's fold input is ready early.
                        nc.vector.tensor_add(
                            out=dtree[:, 0 : 2 * QB],
                            in0=dtree[:, 0 : 2 * QB],
                            in1=dtree[:, 4 * QB : 6 * QB],
                        )
                        nc.vector.tensor_add(
                            out=dtree[:, 0:QB],
                            in0=dtree[:, 0:QB],
                            in1=dtree[:, QB : 2 * QB],
                        )
                        if n > 0:
                            # very last per-iter group: stage on ACT (no exps
                            # left) so the DVE finishes D/E for the final
                            # flush without a 1.2us CAST in front
                            emit_wo_group(
                                n - 1,
                                o_tiles[n - 1],
                                2 * h + 1,
                                on_act=(n == NQ - 1 and h == HL - 1),
                            )
                        pending = (n, h, ps_o, dtree, o_sb)
                    if n > 0:
                        o_tiles.pop(n - 1)
                flush(pending)
                o_last = o_tiles.pop(NQ - 1)
                for g in range(8):
                    # alternate the stage copies ACT/DVE so the final 8
                    # groups' drains pipeline two wide (ACT has no exp left)
                    emit_wo_group(NQ - 1, o_last, g, on_act=(g % 2 == 1))
    nc.compile()
    return nc


def _get_nc():
    if "nc" not in _CACHE:
        _CACHE["nc"] = _build_nc()
    return _CACHE["nc"]


def make_in_maps(query, key_value, Wq, Wk, Wv, Wo):
    import ml_dtypes

    scale = 1.0 / math.sqrt(D)
    f32 = np.float32
    bf16 = ml_dtypes.bfloat16

    def pack_w(w_sl):
        # [HD, DIN] weight slice -> transposed [DIN, HD] -> packed
        # [128, NCH*HD]: partition p row = concat_c w.T[c*128+p, :]
        wT = w_sl.T.astype(bf16)
        return np.ascontiguousarray(
            wT.reshape(NCH, 128, HD).transpose(1, 0, 2).reshape(128, NCH * HD)
        )

    in_maps = []
    qT = [np.ascontiguousarray(query[b].T.astype(bf16)) for b in range(B)]
    kvT = [np.ascontiguousarray(key_value[b].T.astype(bf16)) for b in range(B)]
    for core in range(NC_):
        b, g = divmod(core, NC_ // B)
        sl = slice(g * HD, (g + 1) * HD)
        in_maps.append(
            {
                "qT": qT[b],
                "kvT": kvT[b],
                "wqT": pack_w(Wq[sl, :].astype(f32) * scale),
                "wkT": pack_w(Wk[sl, :]),
                "wvT": pack_w(Wv[sl, :]),
                "woT": np.ascontiguousarray(Wo[:, sl].T.astype(bf16)),
            }
        )
    return in_maps


def _numpy_fallback(query, key_value, attention_mask, Wq, Wk, Wv, Wo):
    # Only reached if the mask is not all-ones (never per the problem spec).
    q64, kv64 = query.astype(np.float64), key_value.astype(np.float64)
    hq = (q64 @ Wq.T.astype(np.float64)).reshape(B, L, NH, D).transpose(0, 2, 1, 3)
    hk = (kv64 @ Wk.T.astype(np.float64)).reshape(B, L, NH, D).transpose(0, 2, 1, 3)
    hv = (kv64 @ Wv.T.astype(np.float64)).reshape(B, L, NH, D).transpose(0, 2, 1, 3)
    s = np.einsum("bhqd,bhkd->bhqk", hq, hk) / math.sqrt(D)
    mask = attention_mask[:, None, :, :]
    s = np.where(mask, s, -np.inf)
    s = s - s.max(axis=-1, keepdims=True)
    e = np.exp(s)
    p = e / np.maximum(e.sum(axis=-1, keepdims=True), 1e-300)
    p = np.where(mask, p, 0.0)
    o = np.einsum("bhqk,bhkd->bhqd", p, hv)
    o = o.transpose(0, 2, 1, 3).reshape(B, L, NH * D)
    return (o @ Wo.T.astype(np.float64)).astype(np.float32)


def kernel(query, key_value, attention_mask, Wq, Wk, Wv, Wo):
    query = np.asarray(query)
    key_value = np.asarray(key_value)
    attention_mask = np.asarray(attention_mask)
    Wq, Wk, Wv, Wo = (np.asarray(a) for a in (Wq, Wk, Wv, Wo))

    if not attention_mask.all():
        return _numpy_fallback(query, key_value, attention_mask, Wq, Wk, Wv, Wo)

    from concourse.bass_utils import run_bass_kernel_spmd

    nc = _get_nc()
    in_maps = make_in_maps(query, key_value, Wq, Wk, Wv, Wo)
    res = run_bass_kernel_spmd(nc, in_maps, list(range(NC_))).results
    out = np.zeros((B, L, DOUT), np.float32)
    for core in range(NC_):
        b = core // (NC_ // B)
        out[b] += res[core]["out"]
    return out
